# revision 27
# baseline (speedup 1.0000x reference)
"""MoE genre-gate kernel for 8 Trainium2 NeuronCores.

Strategy (expert-parallel with token dispatch, per sharding hint):
  - Routing (RMSNorm -> word+genre gate -> softmax -> top-2) is computed on
    host in float64: it is 0.03% of the FLOPs and produces the data-dependent
    dispatch tables (the stand-in for all-to-all).
  - The 8192 (token, expert) pairs are grouped per expert and packed into
    8*SLOTS_PER_CORE fixed-capacity segments (SLOTS_PER_CORE capacity
    classes, one segment of each class per core => identical shapes = SPMD).
    Capacities are chosen by an exact DP that minimizes total padded load;
    the expert weights of each segment are per-core *input data*, so the
    fixed program supports arbitrary expert->segment assignments.  For the
    actual routing this gives 1076 padded tokens/core vs the 1024 ideal
    (5.1% overhead), vs 20% for max-chunk-per-class ranking.
  - Matmuls run in bfloat16 (same PE stream rate as float32r on HW - both
    measured 1 col/cycle - but half the HBM traffic and half the LDWEIGHTS
    time, which must hide under sub-300-col matmul streams).  PSUM
    accumulation is fp32; measured end-to-end error ~5e-3 vs the 2e-2 gate.
  - With zero in-MLP biases (this problem), the host pre-scales each token
    row by its combine weight cw >= 0 (commutes with relu), so padding rows
    are exactly zero and stage 3 runs weight-stationary; a general biased
    fallback path is kept.
  - Stage 3 of slot s-1 is software-interleaved with stage 1 of slot s to
    keep the PE busy across stage boundaries; weight loads are deduplicated
    via chunk-innermost loops + walrus ldw-opt.
  - Host scatter-adds the per-pair outputs back to [B,S,H] and adds the
    (cw @ b3) bias term.

Hardcoded problem shape: B=2, S=2048, H=1024, G=256, E=8, M=2048, top-2.
"""

import numpy as np

import concourse.bass as bass
import concourse.tile as tile
from concourse import mybir
from concourse.bass_utils import run_bass_kernel_spmd
import concourse.bass_utils as _bu

_orig_run_command = _bu.run_command


def _run_command_ldwopt(argv, **kwargs):
    argv = ["--enable-ldw-opt=true" if a == "--enable-ldw-opt=false" else a
            for a in argv]
    return _orig_run_command(argv, **kwargs)


TOP_K = 2
EPS = 1e-6
N_CORES = 8
SLOTS_PER_CORE = 2
H = 1024
M = 2048
KH, KM = H // 128, M // 128
F32R = mybir.dt.float32r
F32 = mybir.dt.float32
BF16 = mybir.dt.bfloat16
MM_DT = BF16          # matmul dtype (same PE rate as float32r, half the DMA)

if MM_DT == F32R:
    # fp32r LDWEIGHTS (191ns) must be deduped across token chunks to hide
    # under the matmul stream; walrus ldw-opt rejects bf16 (FWL) ldweights.
    _bu.run_command = _run_command_ldwopt


# ---------------------------------------------------------------------------
# walrus in this container accepts only ONE sync-wait command per
# instruction; Tile emits up to ~10.  Split extras onto standalone NoOps on
# the same engine, inserted immediately before the instruction, which
# preserves per-engine program order and therefore semantics.
_ctr = [0]


def _legalize_waits(nc, max_waits=1):
    for f in nc.m.functions:
        for blk in f.blocks:
            out = []
            for inst in blk.instructions:
                si = inst.sync_info
                if si is not None and len(si.on_wait) > max_waits:
                    waits = list(si.on_wait)
                    extra, keep = waits[:-max_waits], waits[-max_waits:]
                    for w in extra:
                        _ctr[0] += 1
                        out.append(mybir.InstNoOp(
                            name=f"waitsplit-{_ctr[0]}",
                            engine=inst.engine, ins=[], outs=[],
                            sync_info=mybir.SyncInfo(on_wait=[w], on_update=[]),
                        ))
                    inst.sync_info = mybir.SyncInfo(
                        on_wait=keep, on_update=list(si.on_update))
                out.append(inst)
            blk.instructions = out


# ---------------------------------------------------------------------------
def _batch_mm_updates(nc):
    """Merge per-matmul semaphore increments within runs of consecutive
    InstMatmult on the same engine queue: all-but-last keep their waits but
    drop the increment; the last increments by the run length.  Final counts
    at run boundaries are unchanged; mid-run counts are reached at run end,
    which only ever *delays* a waiter by <1 matmul and cannot deadlock
    (runs never span a PSUM-group boundary's LDWEIGHTS, so no instruction
    inside a run transitively waits on a mid-run count).  Saves the per-MM
    EVT_SEM write cost on the PE queue (~26ns serialized, ~4ns pipelined)."""
    for f in nc.m.functions:
        for blk in f.blocks:
            insts = blk.instructions
            run = []          # indices into insts
            run_sem = None

            def flush():
                if len(run) >= 2:
                    for i in run[:-1]:
                        si = insts[i].sync_info
                        insts[i].sync_info = mybir.SyncInfo(
                            on_wait=list(si.on_wait), on_update=[])
                    last = insts[run[-1]]
                    si = last.sync_info
                    u = si.on_update[0]
                    nu = mybir.SyncUpdate(
                        sync_type=u.sync_type, id=u.id, ant_name=u.ant_name,
                        update_mode=u.update_mode, update_value=len(run),
                        update_reg=u.update_reg)
                    last.sync_info = mybir.SyncInfo(
                        on_wait=list(si.on_wait), on_update=[nu])
                run.clear()

            for i, inst in enumerate(insts):
                si = getattr(inst, 'sync_info', None)
                ok = (isinstance(inst, mybir.InstMatmult) and si is not None
                      and len(si.on_update) == 1
                      and si.on_update[0].update_mode == 'sem-inc'
                      and si.on_update[0].update_value == 1
                      and (run_sem is None or not run
                           or si.on_update[0].id == run_sem))
                if ok:
                    if run and si.on_update[0].id != run_sem:
                        flush()
                    run_sem = si.on_update[0].id
                    run.append(i)
                else:
                    flush()
            flush()


# ---------------------------------------------------------------------------
def _dedup_ldweights(nc):
    """Remove an InstLdweights that reloads the exact same stationary operand
    as the previous InstLdweights on the queue, with only InstMatmult in
    between (the per-token-chunk reload Tile emits when walrus ldw-opt is
    off).  Only sync-free duplicates are removed, so all waits/updates are
    preserved.  The PE keeps the weights in the array across matmuls, which
    is the same contract walrus ldw-opt relies on."""
    removed = 0
    for f in nc.m.functions:
        for blk in f.blocks:
            out = []
            prev_key = None
            for inst in blk.instructions:
                if isinstance(inst, mybir.InstLdweights):
                    si = inst.sync_info
                    key = (str(inst.ins[0]), str(getattr(inst, 'perf_mode', None)),
                           str(getattr(inst, 'is_transpose', None)),
                           str(getattr(inst, 'tile_position', None)))
                    clean = si is None or (not si.on_wait and not si.on_update)
                    if clean and key == prev_key:
                        removed += 1
                        continue
                    prev_key = key
                elif not isinstance(inst, mybir.InstMatmult):
                    prev_key = None
                out.append(inst)
            blk.instructions = out
    return removed


# ---------------------------------------------------------------------------
def _route(x2d, genre_embed, rms_w, wg_W, wg_b, gg_W, gg_b, B, S):
    """Host gating in float64. Returns combine weights [T, E] (zero outside
    top-2)."""
    xd = x2d.astype(np.float64)
    var = np.mean(xd * xd, axis=-1, keepdims=True)
    xn = rms_w.astype(np.float64) * (xd / np.sqrt(var + EPS))
    gate = xn @ wg_W.astype(np.float64) + wg_b.astype(np.float64)
    gg = genre_embed.astype(np.float64)[:, 0, :] @ gg_W.astype(np.float64) \
        + gg_b.astype(np.float64)                       # [B, E]
    gate = gate.reshape(B, S, -1) + gg[:, None, :]
    gate = gate.reshape(B * S, -1)
    gate -= gate.max(axis=-1, keepdims=True)
    p = np.exp(gate)
    p /= p.sum(axis=-1, keepdims=True)
    top2 = np.argsort(-p, axis=-1)[:, :TOP_K]
    cw = np.zeros_like(p)
    rows = np.arange(p.shape[0])[:, None]
    cw[rows, top2] = p[rows, top2]
    return cw.astype(np.float32)


def _pack2(counts, min_cap=224):
    """Pack expert token counts into 8 A-segments + 8 B-segments (A >= B),
    minimizing A+B (the per-core padded load).  Exact: for each candidate A,
    a 9-state DP gives the minimal B-piece total as a function of A-pieces
    used; binary search finds the minimal feasible B.

    Returns (CS, slots): CS = [B, A] slot capacities (small slot first);
    slots[core] = [(expert, lo, hi) or None] * 2 token ranges."""
    E = len(counts)
    hi_cap = max(max(counts), min_cap)
    hi_cap += hi_cap & 1

    def bneed(A, B):
        # dp[j] = minimal total B-pieces using j A-pieces (over experts so far)
        dp = [0] + [10**9] * 8
        for Nc in counts:
            ndp = [10**9] * 9
            for j in range(9):
                if dp[j] >= 10**9:
                    continue
                for na in range(0, 9 - j):
                    rem = Nc - na * A
                    nb = 0 if rem <= 0 else -(-rem // B)
                    if nb > 8:
                        continue
                    v = dp[j] + nb
                    if v < ndp[j + na]:
                        ndp[j + na] = v
            dp = ndp
        return min(dp)

    best = None
    for A in range(hi_cap, min_cap - 1, -2):
        if best is not None and A + min_cap >= best[0]:
            continue   # even the smallest possible B cannot beat best
        lo, hi = min_cap, A
        if bneed(A, hi) > 8:
            continue
        while lo < hi:
            mid = (lo + hi) // 2
            mid -= mid & 1
            if mid < lo:
                mid = lo
            if bneed(A, mid) <= 8:
                hi = mid
            else:
                lo = mid + 2
        B = hi
        if best is None or A + B < best[0]:
            best = (A + B, B, A)
    assert best is not None, "packing failed"
    _, B, A = best

    # reconstruct per-expert piece counts (prefer fewer A-pieces on ties)
    dp = [{0: (0, [])}]
    cur = {0: (0, [])}
    for Nc in counts:
        nxt = {}
        for j, (tb, hist) in cur.items():
            for na in range(0, 9 - j):
                rem = Nc - na * A
                nb = 0 if rem <= 0 else -(-rem // B)
                if nb > 8 or tb + nb > 8:
                    continue
                key = j + na
                if key not in nxt or nxt[key][0] > tb + nb:
                    nxt[key] = (tb + nb, hist + [(na, nb)])
        cur = nxt
    sel = min(cur.values())[1]

    # build pieces: fill A-pieces to capacity first, remainder into B-pieces
    pieces_a, pieces_b = [], []
    for e, (na, nb) in enumerate(sel):
        lo = 0
        n = counts[e]
        for i in range(na):
            take = min(A, n - lo)
            pieces_a.append((e, lo, lo + take))
            lo += take
        for i in range(nb):
            take = min(B, n - lo)
            pieces_b.append((e, lo, lo + take))
            lo += take
        assert lo == n
    while len(pieces_a) < 8:
        pieces_a.append(None)
    while len(pieces_b) < 8:
        pieces_b.append(None)

    # Big slot first: its long per-group compute (~3-5us) easily covers the
    # weight-DMA ramp, and by the time the small slot's marginal 2us-per-
    # group stages run, the DMA queue has a large head start.
    CS = [A, B]
    slots = [[pieces_a[c], pieces_b[c]] for c in range(N_CORES)]
    return CS, slots


def _token_chunks(C):
    """Split C into matmul moving-dim chunks, each <=512 (PSUM bank) and as
    equal as possible."""
    assert C % 2 == 0
    n = -(-C // 512)
    h = C // 2
    base, rem = divmod(h, n)
    sizes = [2 * (base + (1 if i < rem else 0)) for i in range(n)]
    assert sum(sizes) == C and all(s <= 512 and s % 2 == 0 for s in sizes)
    return sizes


# ---------------------------------------------------------------------------
def _build_program(CS, prescaled=False, legalize=True):
    """Emit the SPMD Bass program; CS = per-slot-class capacities.

    prescaled=True (valid when b1==b2==0): host pre-scales x rows by cw
    (cw>=0 commutes with relu), so no bias/cw tiles are needed and stage 3
    runs weight-stationary with output layout [H, C] (y transposed)."""
    S = len(CS)
    nc = bass.Bass()
    # XT is host-packed as [128, KH*C]: column block k holds rows
    # k*128..k*128+127 of x^T.  One SBUF tile per slot, loaded with 4 large
    # column-striped DMAs (2 per HWDGE queue) -- small per-k transfers are
    # completion-serialized per queue and starve stage 1's k-loop at startup.
    xt_d = [nc.dram_tensor(f"XT{s}", [128, KH * CS[s]], MM_DT, kind="ExternalInput") for s in range(S)]
    w1_d = [nc.dram_tensor(f"W1{s}", [KM, 128, H], MM_DT, kind="ExternalInput") for s in range(S)]
    w2_d = [nc.dram_tensor(f"W2{s}", [KM, 128, M], MM_DT, kind="ExternalInput") for s in range(S)]
    if prescaled:
        w3_d = [nc.dram_tensor(f"W3{s}", [KH, 128, M], MM_DT, kind="ExternalInput") for s in range(S)]
        # y in bf16: halves the store traffic and the tail store latency;
        # adds ~2^-9 relative rounding on top of ~3e-3 total (gate is 2e-2)
        y_d = [nc.dram_tensor(f"Y{s}", [H, CS[s]], MM_DT, kind="ExternalOutput") for s in range(S)]
    else:
        w3_d = [nc.dram_tensor(f"W3{s}", [M, H], MM_DT, kind="ExternalInput") for s in range(S)]
        y_d = [nc.dram_tensor(f"Y{s}", [CS[s], H], F32, kind="ExternalOutput") for s in range(S)]
        b1_d = [nc.dram_tensor(f"B1{s}", [M], F32, kind="ExternalInput") for s in range(S)]
        b2_d = [nc.dram_tensor(f"B2{s}", [M], F32, kind="ExternalInput") for s in range(S)]
        cw_d = [nc.dram_tensor(f"CW{s}", [CS[s]], F32, kind="ExternalInput") for s in range(S)]

    HB = 512 if len(CS) >= 3 else 256

    with tile.TileContext(nc) as tc:
        with (
            tc.tile_pool(name="xt", bufs=1) as p_xt,
            tc.tile_pool(name="w1", bufs=6) as p_w1,
            tc.tile_pool(name="w2", bufs=6) as p_w2,
            tc.tile_pool(name="w3", bufs=(6 if prescaled else 8)) as p_w3,
            tc.tile_pool(name="h1", bufs=1) as p_h1,
            tc.tile_pool(name="h2", bufs=1) as p_h2,
            tc.tile_pool(name="bias", bufs=1) as p_b,
            tc.tile_pool(name="y", bufs=4) as p_y,
            tc.tile_pool(name="ps", bufs=8, space="PSUM") as p_ps,
        ):
            st = [dict() for _ in range(S)]   # per-slot tiles/geometry

            def emit_loads(s):
                C = CS[s]
                v = st[s]
                v["tcs"] = _token_chunks(C)
                v["tco"] = np.cumsum([0] + v["tcs"]).tolist()
                v["tts"] = [(i * 128, min(128, C - i * 128)) for i in range(-(-C // 128))]
                v["xta"] = p_xt.tile([128, KH * C], MM_DT, tag="xt", name=f"xt_{s}")
                qw = KH * C // 4
                for q in range(4):
                    eng = nc.sync if q % 2 == 0 else nc.scalar
                    eng.dma_start(out=v["xta"][:, q * qw:(q + 1) * qw],
                                  in_=xt_d[s][:, q * qw:(q + 1) * qw])
                if prescaled:
                    v["h1"] = [p_h1.tile([128, C], MM_DT, tag=f"h1_{m}", name=f"h1_{s}_{m}") for m in range(KM)]
                    v["h2"] = [p_h2.tile([128, C], MM_DT, tag=f"h2_{m}", name=f"h2_{s}_{m}") for m in range(KM)]
                    return
                v["b1t"] = [p_b.tile([128, 1], F32, tag=f"b1_{s}_{m}", name=f"b1t_{s}_{m}") for m in range(KM)]
                v["b2t"] = [p_b.tile([128, 1], F32, tag=f"b2_{s}_{m}", name=f"b2t_{s}_{m}") for m in range(KM)]
                for m in range(KM):
                    nc.scalar.dma_start(out=v["b1t"][m][:], in_=b1_d[s][m * 128:(m + 1) * 128].rearrange("(p one) -> p one", one=1))
                    nc.scalar.dma_start(out=v["b2t"][m][:], in_=b2_d[s][m * 128:(m + 1) * 128].rearrange("(p one) -> p one", one=1))
                v["cwt"] = []
                for t, (t0, tn) in enumerate(v["tts"]):
                    v["cwt"].append(p_b.tile([tn, 1], F32, tag=f"cw_{s}_{t}", name=f"cwt_{s}_{t}"))
                    nc.scalar.dma_start(out=v["cwt"][t][:], in_=cw_d[s][t0:t0 + tn].rearrange("(p one) -> p one", one=1))
                v["h1"] = [p_h1.tile([128, C], MM_DT, tag=f"h1_{m}", name=f"h1_{s}_{m}") for m in range(KM)]
                v["h2"] = [p_h2.tile([128, C], MM_DT, tag=f"h2_{m}", name=f"h2_{s}_{m}") for m in range(KM)]

            def st1_group(s, m):
                v = st[s]
                w1t = p_w1.tile([128, H], MM_DT, tag="w1", name=f"w1t_{s}_{m}")
                nc.gpsimd.dma_start(out=w1t[:], in_=w1_d[s][m])
                pss = [p_ps.tile([128, tcz], F32, tag="ps", name=f"ps1_{s}_{m}_{ci}")
                       for ci, tcz in enumerate(v["tcs"])]
                C = CS[s]
                for k in range(KH):
                    for ci in range(len(v["tcs"])):
                        nc.tensor.matmul(
                            pss[ci][:], w1t[:, k * 128:(k + 1) * 128],
                            v["xta"][:, k * C + v["tco"][ci]:k * C + v["tco"][ci + 1]],
                            start=(k == 0), stop=(k == KH - 1))
                for ci in range(len(v["tcs"])):
                    nc.scalar.activation(
                        v["h1"][m][:, v["tco"][ci]:v["tco"][ci + 1]], pss[ci][:],
                        mybir.ActivationFunctionType.Relu,
                        **({} if prescaled else {"bias": v["b1t"][m][:, 0:1]}))

            def st2_group(s, m):
                v = st[s]
                w2t = p_w2.tile([128, M], MM_DT, tag="w2", name=f"w2t_{s}_{m}")
                nc.gpsimd.dma_start(out=w2t[:], in_=w2_d[s][m])
                pss = [p_ps.tile([128, tcz], F32, tag="ps", name=f"ps2_{s}_{m}_{ci}")
                       for ci, tcz in enumerate(v["tcs"])]
                for k in range(KM):
                    for ci in range(len(v["tcs"])):
                        nc.tensor.matmul(
                            pss[ci][:], w2t[:, k * 128:(k + 1) * 128],
                            v["h1"][k][:, v["tco"][ci]:v["tco"][ci + 1]],
                            start=(k == 0), stop=(k == KM - 1))
                for ci in range(len(v["tcs"])):
                    nc.scalar.activation(
                        v["h2"][m][:, v["tco"][ci]:v["tco"][ci + 1]], pss[ci][:],
                        mybir.ActivationFunctionType.Relu,
                        **({} if prescaled else {"bias": v["b2t"][m][:, 0:1]}))

            def emit_w3(s, hb):
                w3t = [p_w3.tile([128, HB], MM_DT, tag=f"w3_{k % 4}", name=f"w3t_{s}_{hb}_{k}") for k in range(KM)]
                for k in range(KM):
                    nc.gpsimd.dma_start(
                        out=w3t[k][:],
                        in_=w3_d[s][k * 128:(k + 1) * 128, hb * HB:(hb + 1) * HB])
                st[s][f"w3_{hb}"] = w3t

            def st3_group(s, hb, t):
                v = st[s]
                t0, tn = v["tts"][t]
                w3t = v[f"w3_{hb}"]
                ps = p_ps.tile([tn, HB], F32, tag="ps", name=f"ps3_{s}_{hb}_{t}")
                for k in range(KM):
                    nc.tensor.matmul(
                        ps[:], v["h2"][k][:, t0:t0 + tn], w3t[k][:],
                        start=(k == 0), stop=(k == KM - 1))
                yt = p_y.tile([tn, HB], F32, tag="y", name=f"yt_{s}_{hb}_{t}")
                nc.scalar.activation(
                    yt[:], ps[:], mybir.ActivationFunctionType.Copy,
                    scale=v["cwt"][t][:, 0:1])
                nc.scalar.dma_start(
                    out=y_d[s][t0:t0 + tn, hb * HB:(hb + 1) * HB],
                    in_=yt[:])

            def st3_group_ws(s, hm):
                """Weight-stationary stage 3 (prescaled mode): out y_T[h, tok]."""
                v = st[s]
                w3t = p_w3.tile([128, M], MM_DT, tag="w3ws", name=f"w3ws_{s}_{hm}")
                nc.gpsimd.dma_start(out=w3t[:], in_=w3_d[s][hm])
                pss = [p_ps.tile([128, tcz], F32, tag="ps", name=f"ps3_{s}_{hm}_{ci}")
                       for ci, tcz in enumerate(v["tcs"])]
                for k in range(KM):
                    for ci in range(len(v["tcs"])):
                        nc.tensor.matmul(
                            pss[ci][:], w3t[:, k * 128:(k + 1) * 128],
                            v["h2"][k][:, v["tco"][ci]:v["tco"][ci + 1]],
                            start=(k == 0), stop=(k == KM - 1))
                for ci, tcz in enumerate(v["tcs"]):
                    yt = p_y.tile([128, tcz], MM_DT, tag="y", name=f"yt_{s}_{hm}_{ci}")
                    nc.scalar.activation(
                        yt[:], pss[ci][:], mybir.ActivationFunctionType.Copy)
                    nc.scalar.dma_start(
                        out=y_d[s][hm * 128:(hm + 1) * 128, v["tco"][ci]:v["tco"][ci + 1]],
                        in_=yt[:])

            def st3_emitters(s):
                if prescaled:
                    return [lambda s=s, hm=hm: st3_group_ws(s, hm) for hm in range(KH)]
                ems = []
                for hb in range(H // HB):
                    if hb > 0:
                        ems.append(lambda s=s, hb=hb: emit_w3(s, hb))
                    for t in range(len(st[s]["tts"])):
                        ems.append(lambda s=s, hb=hb, t=t: st3_group(s, hb, t))
                return ems

            def interleave(a_ems, b_ems):
                """Emit a and b emitter lists merged evenly (b spread among a)."""
                na, nb = len(a_ems), len(b_ems)
                bi = 0
                for i, a in enumerate(a_ems):
                    while bi < nb and bi * na <= i * nb:
                        b_ems[bi]()
                        bi += 1
                    a()
                while bi < nb:
                    b_ems[bi]()
                    bi += 1

            # ---- PE pre-warm: ~9us of junk matmuls so the HAM clock-gate
            # reaches 2.4 GHz while the first input DMAs land ----
            junk = p_b.tile([128, 512], MM_DT, tag="warm", name="warm_src")
            nc.vector.memset(junk[:], 0.0)   # vector: shortest init preamble
            psw = p_ps.tile([128, 512], F32, tag="ps", name="warm_ps")
            for i in range(10):
                nc.tensor.matmul(psw[:], junk[:, 0:128], junk[:],
                                 start=(i == 0), stop=(i == 9))

            # ---- emission schedule: st3(s-1) interleaves with st1(s) ----
            emit_loads(0)
            prev_st3 = []
            for s in range(S):
                if s > 0:
                    emit_loads(s)
                interleave([lambda s=s, m=m: st1_group(s, m) for m in range(KM)],
                           prev_st3)
                if not prescaled:
                    emit_w3(s, 0)    # prefetch stage-3 hb=0 weights early
                for m in range(KM):
                    st2_group(s, m)
                prev_st3 = st3_emitters(s)
            for em in prev_st3:
                em()

    if legalize:
        _dedup_ldweights(nc)
        # NOTE: batching per-MM sem increments into one +=N update is NOT
        # possible: walrus asserts UpdateValue == 1 on sync_info updates.
        _legalize_waits(nc)
    return nc


# ---------------------------------------------------------------------------
def _run_spmd(CS, prescaled, in_maps):
    """Compile + run on cores 0-7. On a transient device failure (e.g.
    NRT_EXEC_UNIT_UNRECOVERABLE from a stale runtime state), retry in a
    fresh subprocess whose NRT session starts clean."""
    try:
        nc = _build_program(CS, prescaled=prescaled)
        return run_bass_kernel_spmd(nc, in_maps, list(range(N_CORES))).results
    except Exception:
        import os
        import pickle
        import subprocess
        import sys
        import tempfile
        d = tempfile.mkdtemp()
        inp, outp = os.path.join(d, "in.pkl"), os.path.join(d, "out.pkl")
        with open(inp, "wb") as f:
            pickle.dump((CS, prescaled, in_maps), f)
        code = (
            "import pickle, sys\n"
            f"sys.path.insert(0, {os.path.dirname(os.path.abspath(__file__))!r})\n"
            "import kernel as K\n"
            f"CS, prescaled, in_maps = pickle.load(open({inp!r}, 'rb'))\n"
            "nc = K._build_program(CS, prescaled=prescaled)\n"
            "from concourse.bass_utils import run_bass_kernel_spmd\n"
            "r = run_bass_kernel_spmd(nc, in_maps, list(range(K.N_CORES))).results\n"
            f"pickle.dump(r, open({outp!r}, 'wb'))\n"
        )
        err = None
        env = dict(os.environ)
        env["NEURON_RT_RESET_CORES"] = "1"   # recover a wedged device
        for attempt in range(3):
            try:
                subprocess.run([sys.executable, "-c", code], check=True,
                               timeout=1800, env=env)
                with open(outp, "rb") as f:
                    return pickle.load(f)
            except Exception as e:
                err = e
                import time
                time.sleep(5 * (attempt + 1))
        raise err


def kernel(x, genre_embed, rms_w, wg_W, wg_b, gg_W, gg_b, W1, b1, W2, b2, W3, b3):
    x = np.asarray(x, np.float32)
    B, S_, _ = x.shape
    T = B * S_
    x2d = np.ascontiguousarray(x.reshape(T, H))
    W1 = np.asarray(W1, np.float32)
    W2 = np.asarray(W2, np.float32)
    W3 = np.asarray(W3, np.float32)

    if MM_DT == BF16:
        import ml_dtypes
        host_dt = ml_dtypes.bfloat16
    else:
        host_dt = np.float32

    cw = _route(x2d, np.asarray(genre_embed, np.float32), np.asarray(rms_w, np.float32),
                np.asarray(wg_W, np.float32), np.asarray(wg_b, np.float32),
                np.asarray(gg_W, np.float32), np.asarray(gg_b, np.float32), B, S_)
    E = cw.shape[1]
    tok_by_e = [np.nonzero(cw[:, e])[0] for e in range(E)]
    counts = [len(t) for t in tok_by_e]
    CS, slots = _pack2(counts)

    # prescaled mode is exact when the in-MLP biases are zero (cw >= 0
    # commutes with relu); b3 is always applied on the host via cw @ b3
    prescaled = not (np.any(np.asarray(b1)) or np.any(np.asarray(b2)))

    # pre-tile weights once per expert (shared across cores)
    used = set(e for core in slots for p in core if p for e in [p[0]]) or {0}
    w1_tiled, w2_tiled, w3_tiled = {}, {}, {}
    for e in used:
        w1_tiled[e] = np.ascontiguousarray(
            W1[e].reshape(KH, 128, KM, 128).transpose(2, 1, 0, 3).reshape(KM, 128, H)).astype(host_dt)
        w2_tiled[e] = np.ascontiguousarray(
            W2[e].reshape(KM, 128, KM, 128).transpose(2, 1, 0, 3).reshape(KM, 128, M)).astype(host_dt)
        if prescaled:
            w3_tiled[e] = np.ascontiguousarray(
                W3[e].reshape(KM, 128, KH, 128).transpose(2, 1, 0, 3).reshape(KH, 128, M)).astype(host_dt)
        else:
            w3_tiled[e] = W3[e].astype(host_dt)

    in_maps = []
    meta = []
    for core in range(N_CORES):
        im = {}
        cmeta = []
        for si in range(SLOTS_PER_CORE):
            C = CS[si]
            piece = slots[core][si]
            e, lo, hi = piece if piece is not None else (min(used), 0, 0)
            idx = tok_by_e[e][lo:hi]
            n = len(idx)
            xt = np.zeros((H, C), host_dt)
            if prescaled:
                xt[:, :n] = (x2d[idx] * cw[idx, e][:, None]).T.astype(host_dt)
            else:
                xt[:, :n] = x2d[idx].T.astype(host_dt)
            # pack [H, C] -> [128, KH*C]: column block k = rows k*128..+127
            im[f"XT{si}"] = np.ascontiguousarray(
                xt.reshape(KH, 128, C).transpose(1, 0, 2).reshape(128, KH * C))
            im[f"W1{si}"] = w1_tiled[e]
            im[f"W2{si}"] = w2_tiled[e]
            im[f"W3{si}"] = w3_tiled[e]
            if not prescaled:
                cwc = np.zeros((C,), np.float32)
                cwc[:n] = cw[idx, e]
                im[f"B1{si}"] = np.asarray(b1[e], np.float32)
                im[f"B2{si}"] = np.asarray(b2[e], np.float32)
                im[f"CW{si}"] = cwc
            cmeta.append(idx)
        in_maps.append(im)
        meta.append(cmeta)

    results = _run_spmd(CS, prescaled, in_maps)

    out2d = cw @ np.asarray(b3, np.float32)      # bias-3 combine term [T, H]
    for core in range(N_CORES):
        for si, idx in enumerate(meta[core]):
            if len(idx) == 0:
                continue
            y = results[core][f"Y{si}"]
            if prescaled:
                out2d[idx] += np.asarray(y, np.float32)[:, :len(idx)].T
            else:
                out2d[idx] += np.asarray(y, np.float32)[:len(idx)]
    return out2d.reshape(B, S_, H).astype(np.float32)
